# revision 33
# baseline (speedup 1.0000x reference)
"""Bass/Trainium2 kernel for nn_Attention (ragged masked-softmax attention).

Math (per batch b with valid length L):
    c_b      = W_h @ hidden[:, b] + b_attn                  # [2H], W_h = W_attn[:, :H]
    e[s, :]  = tanh(W_e @ x_s + c_b)                        # [2H], W_e = W_attn[:, H:]
    score[s] = w_v . e[s, :] + b_v            (s < L)
    energy   = softmax(score[:L]);  context = energy @ X[:L]

Device strategy: ragged work is packed at 128-position granularity: each
batch's positions form ceil(L/128) sub-chunks; two sub-chunks make one
256-wide device slot.  Same-batch sub-chunks pair first; the per-batch odd
leftovers pair across batches into "mixed" slots, which are placed ONLY at
slot index nchunk-1 — the one slot index the (single, SPMD) program treats
as two independent 128-segments (separate tanh bias / softmax partials /
context rows).  Slots are processed in pairs so the dominant e-matmul runs
as fp8e4 DoubleRow instructions streaming 512 columns (2x PE throughput,
LDWEIGHTS hidden under the stream).  W_e is pre-scaled x512 and X x16 so
fp8 values stay normal; the 1/8192 descale folds into the tanh scale.  The
fp8 quantization error is compensated host-side: a linearized score
correction C[s] = sum_o w_v[o] kappa[o,b] (Delta e)[o,s] (exact residual
algebra against the known quantization errors) is folded into the
per-position mask row.  The c bias columns are computed exactly on host.
Softmax + the context matmul stay fp16/fp32.  Each segment produces
flash-softmax partials (m, Z, ctx) which the host merges exactly.
"""

import numpy as np
import ml_dtypes

import concourse.mybir as mybir
import concourse.tile as tile
from concourse import bacc
from concourse.bass_utils import run_bass_kernel_spmd

B, S, H = 16, 2048, 1024
H2 = 2 * H            # 2048 output features / encoder dim
SUB = 128             # packing granularity (one PE partition block)
CHUNK = 256           # sequence positions per device slot
N_CORES = 8
FB2 = H2 // 256       # 8 fp8 DoubleRow f-blocks (256 features each)
OB = H2 // 128        # 16 o-blocks of the output features
NEG = -30000.0        # masked-score offset (exp underflows to exactly 0)
ALPHA = 512.0         # W_e fp8 pre-scale
BETA = 16.0           # X fp8 pre-scale
ISCL = 1.0 / (ALPHA * BETA)

F8 = mybir.dt.float8e4
F16 = mybir.dt.float16
F32 = mybir.dt.float32
NP_F8 = ml_dtypes.float8_e4m3   # TRN fp8_exp4 (max normal +-240)


def build_program(nchunk: int):
    """nchunk must be even; slot nchunk-1 is handled as two 128-segments."""
    nc = bacc.Bacc()

    npairs = nchunk // 2

    xtp_ext = nc.declare_dram_parameter(
        "xtp", [npairs, 128, FB2, 2, 2 * CHUNK], F8, isOutput=False)
    xn_ext = nc.declare_dram_parameter("xn", [nchunk, 128, CHUNK // 128, H2], F16, isOutput=False)
    mask_ext = nc.declare_dram_parameter("mask", [nchunk, CHUNK], F32, isOutput=False)
    # c bias columns, exact from host, per sub-chunk: cb[p, ob, 2*slot+h]
    c_ext = nc.declare_dram_parameter("cb", [128, OB, 2 * nchunk], F32, isOutput=False)
    wet_ext = nc.declare_dram_parameter("wet", [OB, 128, FB2, 2, 128], F8, isOutput=False)
    wv_ext = nc.declare_dram_parameter("wv", [128, OB], F16, isOutput=False)
    ctx_out = nc.declare_dram_parameter("out_ctx", [2 * nchunk, H2], F32, isOutput=True)
    mz_out = nc.declare_dram_parameter("out_mz", [2 * nchunk, 2], F32, isOutput=True)

    SB = CHUNK // 128   # s-blocks per slot
    DQ = H2 // 512      # 512-wide output quarters for the context matmul
    DR = mybir.MatmulPerfMode.DoubleRow

    def segs_of(i):
        # (col_lo, col_hi, out_idx); slot nchunk-1 is two 128-segments
        if i == nchunk - 1:
            return [(0, 128, 2 * i), (128, 256, 2 * i + 1)]
        return [(0, 256, 2 * i)]

    from contextlib import ExitStack
    with tile.TileContext(nc) as tc, ExitStack() as stk:
        singles = stk.enter_context(tc.tile_pool(name="singles", bufs=1))
        xtp = stk.enter_context(tc.tile_pool(name="xtp", bufs=2))
        xnp = stk.enter_context(tc.tile_pool(name="xnp", bufs=5))
        tp = stk.enter_context(tc.tile_pool(name="tp", bufs=4))
        smalls = stk.enter_context(tc.tile_pool(name="smalls", bufs=3))
        eps = stk.enter_context(tc.tile_pool(name="eps", bufs=2, space="PSUM"))
        sps = stk.enter_context(tc.tile_pool(name="sps", bufs=2, space="PSUM"))
        cps = stk.enter_context(tc.tile_pool(name="cps", bufs=2, space="PSUM"))

        # resident weights as one tile per o-block; xtp0 lands in fb-quarters
        # so the first e-matmul can start after ~0.5MB of DMA
        wet_sb = []
        c_all_sb = singles.tile([128, OB, 2 * nchunk], F32)
        wv_sb = singles.tile([128, OB], F16)
        mask_sb = singles.tile([1, nchunk, CHUNK], F32)
        xt0_sb = xtp.tile([128, FB2, 2, 2 * CHUNK], F8, tag="xt")
        for ob in range(OB):
            w1 = singles.tile([128, FB2, 2, 128], F8, tag=f"wet{ob}")
            if ob == 0:
                # critical path: split the first weight tile and the first
                # xt pair across many DMA queues (each dma_start rides ONE
                # queue at ~27GB/s; a monolithic 256KB tile would gate the
                # first matmul by ~9us)
                for q in range(4):
                    nc.sync.dma_start(out=w1[:, 2 * q:2 * q + 2, :, :],
                                      in_=wet_ext[0, :, 2 * q:2 * q + 2, :, :])
                for q in range(FB2):
                    for half in range(2):
                        nc.sync.dma_start(
                            out=xt0_sb[:, q:q + 1, half:half + 1, :],
                            in_=xtp_ext[0, :, q:q + 1, half:half + 1, :])
                nc.sync.dma_start(out=c_all_sb[:], in_=c_ext[:])
                nc.sync.dma_start(out=wv_sb[:], in_=wv_ext[:])
                nc.sync.dma_start(out=mask_sb[0:1, :, :], in_=mask_ext[:])
            else:
                nc.sync.dma_start(out=w1[:], in_=wet_ext[ob])
            wet_sb.append(w1)
        mz_all = singles.tile([1, 2 * nchunk, 2], F32)
        ident_sb = singles.tile([1, 1], F16)
        nc.vector.memset(ident_sb[:], 1.0)

        def emit_ctx(p):
            i, pt_sb, xn_sb = p[0], p[1], p[2]
            for (lo, hi, oi) in segs_of(i):
                ctx_sb = smalls.tile([1, H2], F32, tag="ctx", name=f"ctx{oi}")
                sbr = range(lo // 128, hi // 128)
                for dq in range(DQ):
                    ctx_ps = cps.tile([1, 512], F32, tag="cps", name=f"cps{oi}_{dq}")
                    for sb in sbr:
                        nc.tensor.matmul(
                            ctx_ps[:],
                            lhsT=pt_sb[:, sb:sb + 1],
                            rhs=xn_sb[:, sb, dq * 512:(dq + 1) * 512],
                            start=(sb == sbr[0]), stop=(sb == sbr[-1]),
                        )
                    if dq % 2 == 0:
                        nc.vector.tensor_copy(out=ctx_sb[0:1, dq * 512:(dq + 1) * 512], in_=ctx_ps[:])
                    else:
                        nc.scalar.copy(out=ctx_sb[0:1, dq * 512:(dq + 1) * 512], in_=ctx_ps[:])
                nc.sync.dma_start(out=ctx_out[oi], in_=ctx_sb[0:1, :])

        def emit_scores(i, t_sb):
            # scores[s] = sum_o w_v[o] t[o, s] -> 4 partial rows (PE column
            # groups; each matmul depends only on its ob's tanh)
            s_ps = sps.tile([128, CHUNK], F32, tag="s", bufs=2)
            for r in range(OB // 4):
                for j in range(4):
                    ob = r * 4 + j
                    nc.tensor.matmul(
                        s_ps[32 * j:32 * j + 1, :],
                        lhsT=wv_sb[:, ob:ob + 1],
                        rhs=t_sb[:, ob, :],
                        start=(r == 0), stop=(r == OB // 4 - 1),
                        tile_position=(0, 32 * j),
                    )
            return s_ps

        def emit_softmax(i, s_ps):
            # masked softmax partials per segment: fold the 4 partial rows +
            # mask (DVE may read at most one PSUM operand per op)
            p_sb = smalls.tile([1, CHUNK], F16, tag="p", bufs=5)
            for (lo, hi, oi) in segs_of(i):
                w = hi - lo
                acc_sb = []
                for j in range(4):
                    prev = mask_sb[0:1, i, lo:hi] if j == 0 else acc_sb[-1][:]
                    a = smalls.tile([1, w], F32, tag=f"fold{j}", name=f"fold{j}_{oi}")
                    nc.vector.tensor_tensor(
                        out=a[:], in0=s_ps[32 * j:32 * j + 1, lo:hi], in1=prev,
                        op=mybir.AluOpType.add,
                    )
                    acc_sb.append(a)
                sc_sb = acc_sb[-1]
                negm_sb = smalls.tile([1, 1], F32, tag="negm", name=f"negm{oi}")
                nc.vector.tensor_reduce(
                    out=negm_sb[:], in_=sc_sb[:],
                    axis=mybir.AxisListType.X, op=mybir.AluOpType.max, negate=True,
                )
                z_sb = smalls.tile([1, 1], F32, tag="z", name=f"z{oi}")
                nc.scalar.activation(
                    out=p_sb[0:1, lo:hi], in_=sc_sb[:],
                    func=mybir.ActivationFunctionType.Exp,
                    bias=negm_sb[0:1, :], scale=1.0, accum_out=z_sb[:],
                )
                nc.vector.tensor_copy(out=mz_all[0:1, oi, 0:1], in_=negm_sb[:])
                nc.vector.tensor_copy(out=mz_all[0:1, oi, 1:2], in_=z_sb[:])
            xn_sb = xnp.tile([128, SB, H2], F16, tag="xn")
            nc.sync.dma_start(out=xn_sb[:], in_=xn_ext[i])
            return [i, p_sb, xn_sb]

        def emit_pt(p):
            # p row -> column layout [128, SB] via PE transpose.  Deferred to
            # the NEXT group's PE stream so it never waits on the softmax
            # chain.
            i, p_sb = p[0], p[1]
            pt_sb = smalls.tile([128, SB], F16, tag="pt", bufs=4)
            for sb in range(SB):
                t_ps = sps.tile([128, 1], F16, tag="tp", bufs=2)
                nc.tensor.transpose(
                    t_ps[:], p_sb[0:1, sb * 128:(sb + 1) * 128], ident_sb[:])
                nc.vector.tensor_copy(out=pt_sb[:, sb:sb + 1], in_=t_ps[:])
            p[1] = pt_sb

        pending = []
        for g in range(npairs):
            chunks = [2 * g, 2 * g + 1]
            if g == 0:
                xt_sb = xt0_sb
            else:
                xt_sb = xtp.tile([128, FB2, 2, 2 * CHUNK], F8, tag="xt")
                nc.sync.dma_start(out=xt_sb[:], in_=xtp_ext[g])

            t_list = [tp.tile([128, OB, CHUNK], F16, tag="t", name=f"t{g}_{h}")
                      for h in range(2)]
            for ob in range(OB):
                e_ps = eps.tile([128, 2 * CHUNK], F32, tag="e")
                for fb in range(FB2):
                    nc.tensor.matmul(
                        e_ps[:],
                        lhsT=wet_sb[ob][:, fb, :, :],
                        rhs=xt_sb[:, fb, :, :],
                        start=(fb == 0), stop=(fb == FB2 - 1),
                        perf_mode=DR,
                    )
                for h, i in enumerate(chunks):
                    for (lo, hi, oi) in segs_of(i):
                        nc.scalar.activation(
                            out=t_list[h][:, ob, lo:hi],
                            in_=e_ps[:, h * CHUNK + lo:h * CHUNK + hi],
                            func=mybir.ActivationFunctionType.Tanh,
                            bias=c_all_sb[:, ob, oi:oi + 1], scale=ISCL,
                        )

            for h, i in enumerate(chunks):
                # drain only chunks from a PREVIOUS group so pt/ctx never
                # wait on a softmax chain that just issued
                drain = bool(pending) and pending[0][3] < g - 1
                s_ps = emit_scores(i, t_list[h])
                if drain:
                    emit_pt(pending[0])
                    emit_ctx(pending.pop(0))
                pending.append(emit_softmax(i, s_ps) + [g])

        nc.sync.dma_start(out=mz_out[:], in_=mz_all[0:1, :, :])
        while pending:
            p = pending.pop(0)
            emit_pt(p)
            emit_ctx(p)

    nc.compile()
    return nc


def kernel(encoder_out, hidden, W_attn, b_attn, w_v, b_v, lengths):
    encoder_out = np.asarray(encoder_out)
    hidden = np.asarray(hidden)
    W_attn = np.asarray(W_attn)
    b_attn = np.asarray(b_attn)
    w_v = np.asarray(w_v)
    b_v = np.asarray(b_v)
    lengths = np.asarray(lengths)

    # ---- host-side 128-granular packing from the runtime lengths ----
    # sub-chunk: (batch, s0, v<=128); slot: two sub-chunks
    same_slots = []   # [(sub, sub)] both from one batch
    leftovers = []
    for b in range(B):
        L = int(lengths[b])
        subs = [(b, s0, min(SUB, L - s0)) for s0 in range(0, L, SUB)]
        for j in range(len(subs) // 2):
            same_slots.append((subs[2 * j], subs[2 * j + 1]))
        if len(subs) % 2:
            leftovers.append(subs[-1])
    mixed_slots = []
    while len(leftovers) >= 2 and len(mixed_slots) < N_CORES:
        mixed_slots.append((leftovers.pop(0), leftovers.pop(0)))
    # any remaining leftover becomes a half-empty same-batch slot
    for lf in leftovers:
        same_slots.append((lf, None))

    nslots = len(same_slots) + len(mixed_slots)
    nchunk = max(2, -(-nslots // N_CORES))
    if nchunk % 2:
        nchunk += 1
    # mixed slots only fit at index nchunk-1 (one per core)
    while len(mixed_slots) > N_CORES or \
            len(same_slots) > N_CORES * (nchunk - 1) + (N_CORES - len(mixed_slots)):
        nchunk += 2

    # per-core slot assignment: indices 0..nchunk-2 from same_slots; index
    # nchunk-1 takes mixed first, then remaining same, then padding
    core_slots = [[None] * nchunk for _ in range(N_CORES)]
    it_same = iter(same_slots)
    filled = 0
    for c in range(N_CORES):
        for i in range(nchunk - 1):
            core_slots[c][i] = next(it_same, None)
    for c in range(N_CORES):
        if c < len(mixed_slots):
            core_slots[c][nchunk - 1] = mixed_slots[c]
        else:
            core_slots[c][nchunk - 1] = next(it_same, None)
    assert next(it_same, None) is None

    We = W_attn[:, H:]                          # [2H, 2H]
    Wh = W_attn[:, :H]                          # [2H, H]

    # ---- fp8 weight quantization (+ residual for the score correction) ----
    W8q = (We * ALPHA).astype(NP_F8)            # [o, f] fp8 payload
    W8f = W8q.astype(np.float32)
    dW = We * ALPHA - W8f                       # exact residual (host)

    # wet[ob, p, fb, i, q] = W8[ob*128+q, fb*256 + i*128 + p]
    wet = np.ascontiguousarray(
        W8q.reshape(OB, 128, FB2, 2, 128).transpose(0, 4, 2, 3, 1))
    wv = np.ascontiguousarray(w_v[0].reshape(OB, 128).T).astype(np.float16)

    # ---- linearized fp8 score correction (host, exact residual algebra) ----
    # kappa[o, b] = E_s[1 - tanh^2(e8[o,s])] with e8 ~ N(c[o,b], sigma_o^2)
    c_all = (Wh @ hidden) + b_attn[:, None]     # [2H, B]
    sig = np.linalg.norm(We, axis=1)            # [2H]
    gh_x, gh_w = np.polynomial.hermite_e.hermegauss(8)
    gh_w = (gh_w / gh_w.sum()).astype(np.float64)
    z = c_all[:, None, :] + sig[:, None, None] * gh_x[None, :, None]
    kappa = np.einsum("okb,k->ob", 1.0 / np.cosh(z) ** 2, gh_w,
                      optimize=True).astype(np.float32)   # [2H, B]
    wk = w_v[0][:, None] * kappa                # [2H_o, B]
    g_all = dW.T @ wk                           # [2H_f, B]
    h8_all = W8f.T @ wk                         # [2H_f, B]

    x16 = encoder_out.astype(np.float16)
    x8 = np.empty((B, S, H2), NP_F8)
    corr = np.empty((B, S), np.float32)
    for b in range(B):
        x8[b] = (encoder_out[b] * BETA).astype(NP_F8)
        x8f = x8[b].astype(np.float32)          # [S, 2H]
        dX = encoder_out[b] * BETA - x8f
        corr[b] = (BETA * (encoder_out[b] @ g_all[:, b])
                   + dX @ h8_all[:, b]) * ISCL

    nc = build_program(nchunk)

    npairs = nchunk // 2
    c_dev = c_all.reshape(OB, 128, B)           # [ob, p, B]

    # segment records for the host flash merge: (batch, core, out_idx)
    seg_recs = []
    in_maps = []
    for c in range(N_CORES):
        xtp_a = np.zeros((npairs, 128, FB2, 2, 2 * CHUNK), NP_F8)
        xn = np.zeros((nchunk, 128, CHUNK // 128, H2), np.float16)
        mask = np.full((nchunk, CHUNK), NEG + float(b_v[0]), np.float32)
        cb = np.zeros((128, OB, 2 * nchunk), np.float32)
        for i in range(nchunk):
            slot = core_slots[c][i]
            if slot is None:
                continue
            two_seg = (i == nchunk - 1)
            for h, sub in enumerate(slot):
                if sub is None:
                    continue
                b, s0, v = sub
                chunk8 = x8[b, s0:s0 + v, :]                 # [v, 2048] fp8
                xt_block = np.zeros((128, FB2, 2, SUB), NP_F8)
                xt_block[:, :, :, :v] = chunk8.reshape(v, FB2, 2, 128).transpose(3, 1, 2, 0)
                xtp_a[i // 2, :, :, :, (i % 2) * CHUNK + h * SUB:
                      (i % 2) * CHUNK + (h + 1) * SUB] = xt_block
                full = np.zeros((SUB, H2), np.float16)
                full[:v] = x16[b, s0:s0 + v, :]
                xn[i, :, h, :] = full.reshape(SUB, H2)
                mask[i, h * SUB:h * SUB + v] = float(b_v[0]) + corr[b, s0:s0 + v]
                cb[:, :, 2 * i + h] = c_dev[:, :, b].T
            if two_seg:
                for h, sub in enumerate(slot):
                    if sub is not None:
                        seg_recs.append((sub[0], c, 2 * i + h))
            else:
                b0 = slot[0][0]
                cb[:, :, 2 * i + 1] = c_dev[:, :, b0].T  # unused but sane
                seg_recs.append((b0, c, 2 * i))
        in_maps.append(dict(xtp=xtp_a, xn=xn, mask=mask, cb=cb, wet=wet, wv=wv))

    def run_once():
        res = run_bass_kernel_spmd(nc, in_maps, core_ids=list(range(N_CORES)))
        negm = np.stack([res.results[c]["out_mz"][:, 0] for c in range(N_CORES)])
        zz = np.stack([res.results[c]["out_mz"][:, 1] for c in range(N_CORES)])
        ctx = np.stack([res.results[c]["out_ctx"] for c in range(N_CORES)])
        return negm, zz, ctx

    def merge(parts):
        negm, zz, ctx = parts
        # ---- exact flash-softmax merge on host ----
        out = np.zeros((B, H2), np.float32)
        ok = True
        for b in range(B):
            idxs = [(c, oi) for (ub, c, oi) in seg_recs if ub == b]
            ms = np.array([-float(negm[c, oi]) for c, oi in idxs])
            if not np.isfinite(ms).all():
                ok = False
                ms = np.nan_to_num(ms, nan=-np.inf)
            m = ms.max()
            w = np.exp(ms - m)
            Z = float(sum(wi * float(zz[c, oi]) for wi, (c, oi) in zip(w, idxs)))
            if not (Z > 0 and np.isfinite(Z)):
                ok = False
                Z = 1.0
            acc = np.zeros(H2, np.float64)
            for wi, (c, oi) in zip(w, idxs):
                acc += wi * ctx[c, oi].astype(np.float64)
            out[b] = (acc / Z).astype(np.float32)
        # context rows are convex combinations of encoder_out rows
        ok = ok and np.isfinite(out).all() and np.abs(out).max() < 50.0
        return out, ok

    out, ok = merge(run_once())
    if not ok:  # one retry on gross corruption
        out, ok = merge(run_once())
    return out


# revision 48
# speedup vs baseline: 1.1293x; 1.1293x over previous
"""Bass/Trainium2 kernel for nn_Attention (ragged masked-softmax attention).

Math (per batch b with valid length L):
    c_b      = W_h @ hidden[:, b] + b_attn                  # [2H], W_h = W_attn[:, :H]
    e[s, :]  = tanh(W_e @ x_s + c_b)                        # [2H], W_e = W_attn[:, H:]
    score[s] = w_v . e[s, :] + b_v            (s < L)
    energy   = softmax(score[:L]);  context = energy @ X[:L]

Device strategy: ragged work is packed at 128-position granularity: each
batch's positions form ceil(L/128) sub-chunks; two sub-chunks make one
256-wide device slot.  Same-batch sub-chunks pair first; the per-batch odd
leftovers pair across batches into "mixed" slots, which are placed ONLY at
slot index nchunk-1 — the one slot index the (single, SPMD) program treats
as two independent 128-segments (separate tanh bias / softmax partials /
context rows).  Slots are processed in pairs so the dominant e-matmul runs
as fp8e4 DoubleRow instructions streaming 512 columns (2x PE throughput,
LDWEIGHTS hidden under the stream).  W_e is pre-scaled x512 and X x16 so
fp8 values stay normal; the 1/8192 descale folds into the tanh scale.  The
fp8 quantization error is compensated host-side: a linearized score
correction C[s] = sum_o w_v[o] kappa[o,b] (Delta e)[o,s] (exact residual
algebra against the known quantization errors) is folded into the
per-position mask row.  The c bias columns are computed exactly on host.
Softmax + the context matmul stay fp16/fp32.  Each segment produces
flash-softmax partials (m, Z, ctx) which the host merges exactly.
"""

import numpy as np
import ml_dtypes

import concourse.mybir as mybir
import concourse.tile as tile
from concourse import bacc
from concourse.bass_utils import run_bass_kernel_spmd

B, S, H = 16, 2048, 1024
H2 = 2 * H            # 2048 output features / encoder dim
SUB = 128             # packing granularity (one PE partition block)
CHUNK = 256           # sequence positions per device slot
N_CORES = 8
FB2 = H2 // 256       # 8 fp8 DoubleRow f-blocks (256 features each)
OB = H2 // 128        # 16 o-blocks of the output features
NEG = -30000.0        # masked-score offset (exp underflows to exactly 0)
ALPHA = 512.0         # W_e fp8 pre-scale
BETA = 16.0           # X fp8 pre-scale
ISCL = 1.0 / (ALPHA * BETA)

F8 = mybir.dt.float8e4
F16 = mybir.dt.float16
F32 = mybir.dt.float32
NP_F8 = ml_dtypes.float8_e4m3   # TRN fp8_exp4 (max normal +-240)


def build_program(nchunk: int):
    """nchunk must be even; slot nchunk-1 is handled as two 128-segments."""
    nc = bacc.Bacc()

    npairs = nchunk // 2

    xtp_ext = nc.declare_dram_parameter(
        "xtp", [npairs, 128, FB2, 2, 2 * CHUNK], F8, isOutput=False)
    # xc: fp16 X in feature-major layout for the DVE context reduction:
    # xc[slot, p, fb, s] = X[s, fb*128 + p]
    xc_ext = nc.declare_dram_parameter("xc", [nchunk, 128, OB, CHUNK], F16, isOutput=False)
    mask_ext = nc.declare_dram_parameter("mask", [nchunk, CHUNK], F32, isOutput=False)
    # c bias columns, exact from host, per sub-chunk: cb[p, ob, 2*slot+h]
    c_ext = nc.declare_dram_parameter("cb", [128, OB, 2 * nchunk], F32, isOutput=False)
    wet_ext = nc.declare_dram_parameter("wet", [OB, 128, FB2, 2, 128], F8, isOutput=False)
    wv_ext = nc.declare_dram_parameter("wv", [128, OB], F16, isOutput=False)
    # ctxT[oi, p, fb] = context[fb*128 + p] for segment oi
    ctx_out = nc.declare_dram_parameter("out_ctx", [2 * nchunk, 128, OB], F32, isOutput=True)
    mz_out = nc.declare_dram_parameter("out_mz", [2 * nchunk, 2], F32, isOutput=True)

    DR = mybir.MatmulPerfMode.DoubleRow

    def segs_of(i):
        # (col_lo, col_hi, out_idx); slot nchunk-1 is two 128-segments
        if i == nchunk - 1:
            return [(0, 128, 2 * i), (128, 256, 2 * i + 1)]
        return [(0, 256, 2 * i)]

    from contextlib import ExitStack
    with tile.TileContext(nc) as tc, ExitStack() as stk:
        singles = stk.enter_context(tc.tile_pool(name="singles", bufs=1))
        xtp = stk.enter_context(tc.tile_pool(name="xtp", bufs=2))
        xcp = stk.enter_context(tc.tile_pool(name="xcp", bufs=4))
        tp = stk.enter_context(tc.tile_pool(name="tp", bufs=4))
        smalls = stk.enter_context(tc.tile_pool(name="smalls", bufs=3))
        eps = stk.enter_context(tc.tile_pool(name="eps", bufs=3, space="PSUM"))
        sps = stk.enter_context(tc.tile_pool(name="sps", bufs=2, space="PSUM"))

        # resident weights as one tile per o-block; xtp0 lands in fb-quarters
        # so the first e-matmul can start after ~0.5MB of DMA
        wet_sb = []
        c_all_sb = singles.tile([128, OB, 2 * nchunk], F32)
        wv_sb = singles.tile([128, OB], F16)
        mask_sb = singles.tile([1, nchunk, CHUNK], F32)
        xt0_sb = xtp.tile([128, FB2, 2, 2 * CHUNK], F8, tag="xt")
        for ob in range(OB):
            if ob == 0:
                nc.sync.dma_start(out=c_all_sb[:], in_=c_ext[:])
            w1 = singles.tile([128, FB2, 2, 128], F8, tag=f"wet{ob}")
            nc.sync.dma_start(out=w1[:], in_=wet_ext[ob])
            wet_sb.append(w1)
            if ob == 0:
                for q in range(FB2):
                    nc.sync.dma_start(
                        out=xt0_sb[:, q:q + 1, :, :],
                        in_=xtp_ext[0, :, q:q + 1, :, :])
                nc.sync.dma_start(out=wv_sb[:], in_=wv_ext[:])
                nc.sync.dma_start(out=mask_sb[0:1, :, :], in_=mask_ext[:])
        mz_all = singles.tile([1, 2 * nchunk, 2], F32)

        def emit_ctx(i, p_sb, xc_sb):
            # context on GpSimd+DVE: broadcast the energy row, then one
            # fused multiply+reduce per 128-feature block (PE not involved)
            p_bc = smalls.tile([128, CHUNK], F16, tag="pbc", name=f"pbc{i}")
            nc.gpsimd.partition_broadcast(p_bc[:], p_sb[0:1, :])
            for (lo, hi, oi) in segs_of(i):
                ctxT = smalls.tile([128, OB], F32, tag="ctxT", name=f"ctxT{oi}")
                for fb in range(OB):
                    junk = smalls.tile([128, hi - lo], F16, tag="junk",
                                       name=f"junk{oi}_{fb}", bufs=2)
                    nc.vector.tensor_tensor(
                        out=junk[:], in0=xc_sb[:, fb, lo:hi], in1=p_bc[:, lo:hi],
                        op=mybir.AluOpType.mult,
                    )
                    nc.vector.tensor_reduce(
                        out=ctxT[:, fb:fb + 1], in_=junk[:],
                        axis=mybir.AxisListType.X, op=mybir.AluOpType.add,
                    )
                nc.sync.dma_start(out=ctx_out[oi], in_=ctxT[:])

        def emit_scores(i, t_sb):
            # scores[s] = sum_o w_v[o] t[o, s] -> 4 partial rows (PE column
            # groups; each matmul depends only on its ob's tanh)
            s_ps = sps.tile([128, CHUNK], F32, tag="s", bufs=2)
            for r in range(OB // 4):
                for j in range(4):
                    ob = r * 4 + j
                    nc.tensor.matmul(
                        s_ps[32 * j:32 * j + 1, :],
                        lhsT=wv_sb[:, ob:ob + 1],
                        rhs=t_sb[:, ob, :],
                        start=(r == 0), stop=(r == OB // 4 - 1),
                        tile_position=(0, 32 * j),
                    )
            return s_ps

        def emit_softmax(i, s_ps):
            # masked softmax partials per segment: fold the 4 partial rows +
            # mask (DVE may read at most one PSUM operand per op)
            p_sb = smalls.tile([1, CHUNK], F16, tag="p")
            for (lo, hi, oi) in segs_of(i):
                w = hi - lo
                acc_sb = []
                for j in range(4):
                    prev = mask_sb[0:1, i, lo:hi] if j == 0 else acc_sb[-1][:]
                    a = smalls.tile([1, w], F32, tag=f"fold{j}", name=f"fold{j}_{oi}")
                    nc.vector.tensor_tensor(
                        out=a[:], in0=s_ps[32 * j:32 * j + 1, lo:hi], in1=prev,
                        op=mybir.AluOpType.add,
                    )
                    acc_sb.append(a)
                sc_sb = acc_sb[-1]
                negm_sb = smalls.tile([1, 1], F32, tag="negm", name=f"negm{oi}")
                nc.vector.tensor_reduce(
                    out=negm_sb[:], in_=sc_sb[:],
                    axis=mybir.AxisListType.X, op=mybir.AluOpType.max, negate=True,
                )
                z_sb = smalls.tile([1, 1], F32, tag="z", name=f"z{oi}")
                nc.scalar.activation(
                    out=p_sb[0:1, lo:hi], in_=sc_sb[:],
                    func=mybir.ActivationFunctionType.Exp,
                    bias=negm_sb[0:1, :], scale=1.0, accum_out=z_sb[:],
                )
                nc.vector.tensor_copy(out=mz_all[0:1, oi, 0:1], in_=negm_sb[:])
                nc.vector.tensor_copy(out=mz_all[0:1, oi, 1:2], in_=z_sb[:])
            return p_sb

        for g in range(npairs):
            chunks = [2 * g, 2 * g + 1]
            if g == 0:
                xt_sb = xt0_sb
            else:
                xt_sb = xtp.tile([128, FB2, 2, 2 * CHUNK], F8, tag="xt")
                nc.sync.dma_start(out=xt_sb[:], in_=xtp_ext[g])
            xc_list = []
            for h, i in enumerate(chunks):
                xc_sb = xcp.tile([128, OB, CHUNK], F16, tag="xc", name=f"xc{g}_{h}")
                nc.sync.dma_start(out=xc_sb[:], in_=xc_ext[i])
                xc_list.append(xc_sb)

            t_list = [tp.tile([128, OB, CHUNK], F16, tag="t", name=f"t{g}_{h}")
                      for h in range(2)]
            for ob in range(OB):
                e_ps = eps.tile([128, 2 * CHUNK], F32, tag="e")
                for fb in range(FB2):
                    nc.tensor.matmul(
                        e_ps[:],
                        lhsT=wet_sb[ob][:, fb, :, :],
                        rhs=xt_sb[:, fb, :, :],
                        start=(fb == 0), stop=(fb == FB2 - 1),
                        perf_mode=DR,
                    )
                for h, i in enumerate(chunks):
                    for (lo, hi, oi) in segs_of(i):
                        nc.scalar.activation(
                            out=t_list[h][:, ob, lo:hi],
                            in_=e_ps[:, h * CHUNK + lo:h * CHUNK + hi],
                            func=mybir.ActivationFunctionType.Tanh,
                            bias=c_all_sb[:, ob, oi:oi + 1], scale=ISCL,
                        )

            for h, i in enumerate(chunks):
                s_ps = emit_scores(i, t_list[h])
                p_sb = emit_softmax(i, s_ps)
                emit_ctx(i, p_sb, xc_list[h])

        nc.sync.dma_start(out=mz_out[:], in_=mz_all[0:1, :, :])

    nc.compile()
    return nc


def kernel(encoder_out, hidden, W_attn, b_attn, w_v, b_v, lengths):
    encoder_out = np.asarray(encoder_out)
    hidden = np.asarray(hidden)
    W_attn = np.asarray(W_attn)
    b_attn = np.asarray(b_attn)
    w_v = np.asarray(w_v)
    b_v = np.asarray(b_v)
    lengths = np.asarray(lengths)

    # ---- host-side 128-granular packing from the runtime lengths ----
    # sub-chunk: (batch, s0, v<=128); slot: two sub-chunks
    same_slots = []   # [(sub, sub)] both from one batch
    leftovers = []
    for b in range(B):
        L = int(lengths[b])
        subs = [(b, s0, min(SUB, L - s0)) for s0 in range(0, L, SUB)]
        for j in range(len(subs) // 2):
            same_slots.append((subs[2 * j], subs[2 * j + 1]))
        if len(subs) % 2:
            leftovers.append(subs[-1])
    mixed_slots = []
    while len(leftovers) >= 2 and len(mixed_slots) < N_CORES:
        mixed_slots.append((leftovers.pop(0), leftovers.pop(0)))
    # any remaining leftover becomes a half-empty same-batch slot
    for lf in leftovers:
        same_slots.append((lf, None))

    nslots = len(same_slots) + len(mixed_slots)
    nchunk = max(2, -(-nslots // N_CORES))
    if nchunk % 2:
        nchunk += 1
    # mixed slots only fit at index nchunk-1 (one per core)
    while len(mixed_slots) > N_CORES or \
            len(same_slots) > N_CORES * (nchunk - 1) + (N_CORES - len(mixed_slots)):
        nchunk += 2

    # per-core slot assignment: indices 0..nchunk-2 from same_slots; index
    # nchunk-1 takes mixed first, then remaining same, then padding
    core_slots = [[None] * nchunk for _ in range(N_CORES)]
    it_same = iter(same_slots)
    filled = 0
    for c in range(N_CORES):
        for i in range(nchunk - 1):
            core_slots[c][i] = next(it_same, None)
    for c in range(N_CORES):
        if c < len(mixed_slots):
            core_slots[c][nchunk - 1] = mixed_slots[c]
        else:
            core_slots[c][nchunk - 1] = next(it_same, None)
    assert next(it_same, None) is None

    We = W_attn[:, H:]                          # [2H, 2H]
    Wh = W_attn[:, :H]                          # [2H, H]

    # ---- fp8 weight quantization (+ residual for the score correction) ----
    W8q = (We * ALPHA).astype(NP_F8)            # [o, f] fp8 payload
    W8f = W8q.astype(np.float32)
    dW = We * ALPHA - W8f                       # exact residual (host)

    # wet[ob, p, fb, i, q] = W8[ob*128+q, fb*256 + i*128 + p]
    wet = np.ascontiguousarray(
        W8q.reshape(OB, 128, FB2, 2, 128).transpose(0, 4, 2, 3, 1))
    wv = np.ascontiguousarray(w_v[0].reshape(OB, 128).T).astype(np.float16)

    # ---- linearized fp8 score correction (host, exact residual algebra) ----
    # kappa[o, b] = E_s[1 - tanh^2(e8[o,s])] with e8 ~ N(c[o,b], sigma_o^2)
    c_all = (Wh @ hidden) + b_attn[:, None]     # [2H, B]
    sig = np.linalg.norm(We, axis=1)            # [2H]
    gh_x, gh_w = np.polynomial.hermite_e.hermegauss(8)
    gh_w = (gh_w / gh_w.sum()).astype(np.float64)
    z = c_all[:, None, :] + sig[:, None, None] * gh_x[None, :, None]
    kappa = np.einsum("okb,k->ob", 1.0 / np.cosh(z) ** 2, gh_w,
                      optimize=True).astype(np.float32)   # [2H, B]
    wk = w_v[0][:, None] * kappa                # [2H_o, B]
    g_all = dW.T @ wk                           # [2H_f, B]
    h8_all = W8f.T @ wk                         # [2H_f, B]

    x16 = encoder_out.astype(np.float16)
    x8 = np.empty((B, S, H2), NP_F8)
    corr = np.empty((B, S), np.float32)
    for b in range(B):
        x8[b] = (encoder_out[b] * BETA).astype(NP_F8)
        x8f = x8[b].astype(np.float32)          # [S, 2H]
        dX = encoder_out[b] * BETA - x8f
        corr[b] = (BETA * (encoder_out[b] @ g_all[:, b])
                   + dX @ h8_all[:, b]) * ISCL

    nc = build_program(nchunk)

    npairs = nchunk // 2
    c_dev = c_all.reshape(OB, 128, B)           # [ob, p, B]

    # segment records for the host flash merge: (batch, core, out_idx)
    seg_recs = []
    in_maps = []
    for c in range(N_CORES):
        xtp_a = np.zeros((npairs, 128, FB2, 2, 2 * CHUNK), NP_F8)
        xc = np.zeros((nchunk, 128, OB, CHUNK), np.float16)
        mask = np.full((nchunk, CHUNK), NEG + float(b_v[0]), np.float32)
        cb = np.zeros((128, OB, 2 * nchunk), np.float32)
        for i in range(nchunk):
            slot = core_slots[c][i]
            if slot is None:
                continue
            two_seg = (i == nchunk - 1)
            for h, sub in enumerate(slot):
                if sub is None:
                    continue
                b, s0, v = sub
                chunk8 = x8[b, s0:s0 + v, :]                 # [v, 2048] fp8
                xt_block = np.zeros((128, FB2, 2, SUB), NP_F8)
                xt_block[:, :, :, :v] = chunk8.reshape(v, FB2, 2, 128).transpose(3, 1, 2, 0)
                xtp_a[i // 2, :, :, :, (i % 2) * CHUNK + h * SUB:
                      (i % 2) * CHUNK + (h + 1) * SUB] = xt_block
                full = np.zeros((SUB, H2), np.float16)
                full[:v] = x16[b, s0:s0 + v, :]
                # xc[p, fb, s] = X[s, fb*128 + p]
                xc[i, :, :, h * SUB:h * SUB + SUB] = \
                    full.reshape(SUB, OB, 128).transpose(2, 1, 0)
                mask[i, h * SUB:h * SUB + v] = float(b_v[0]) + corr[b, s0:s0 + v]
                cb[:, :, 2 * i + h] = c_dev[:, :, b].T
            if two_seg:
                for h, sub in enumerate(slot):
                    if sub is not None:
                        seg_recs.append((sub[0], c, 2 * i + h))
            else:
                b0 = slot[0][0]
                cb[:, :, 2 * i + 1] = c_dev[:, :, b0].T  # unused but sane
                seg_recs.append((b0, c, 2 * i))
        in_maps.append(dict(xtp=xtp_a, xc=xc, mask=mask, cb=cb, wet=wet, wv=wv))

    def run_once():
        res = run_bass_kernel_spmd(nc, in_maps, core_ids=list(range(N_CORES)))
        negm = np.stack([res.results[c]["out_mz"][:, 0] for c in range(N_CORES)])
        zz = np.stack([res.results[c]["out_mz"][:, 1] for c in range(N_CORES)])
        ctx = np.stack([res.results[c]["out_ctx"] for c in range(N_CORES)])
        return negm, zz, ctx

    def merge(parts):
        negm, zz, ctx = parts
        # ---- exact flash-softmax merge on host ----
        out = np.zeros((B, H2), np.float32)
        ok = True
        for b in range(B):
            idxs = [(c, oi) for (ub, c, oi) in seg_recs if ub == b]
            ms = np.array([-float(negm[c, oi]) for c, oi in idxs])
            if not np.isfinite(ms).all():
                ok = False
                ms = np.nan_to_num(ms, nan=-np.inf)
            m = ms.max()
            w = np.exp(ms - m)
            Z = float(sum(wi * float(zz[c, oi]) for wi, (c, oi) in zip(w, idxs)))
            if not (Z > 0 and np.isfinite(Z)):
                ok = False
                Z = 1.0
            acc = np.zeros(H2, np.float64)
            for wi, (c, oi) in zip(w, idxs):
                # ctx layout [128 p, OB fb]: feature f = fb*128 + p
                acc += wi * ctx[c, oi].T.reshape(H2).astype(np.float64)
            out[b] = (acc / Z).astype(np.float32)
        # context rows are convex combinations of encoder_out rows
        ok = ok and np.isfinite(out).all() and np.abs(out).max() < 50.0
        return out, ok

    out, ok = merge(run_once())
    if not ok:  # one retry on gross corruption
        out, ok = merge(run_once())
    return out


# revision 52
# speedup vs baseline: 1.1530x; 1.0210x over previous
"""Bass/Trainium2 kernel for nn_Attention (ragged masked-softmax attention).

Math (per batch b with valid length L):
    c_b      = W_h @ hidden[:, b] + b_attn                  # [2H], W_h = W_attn[:, :H]
    e[s, :]  = tanh(W_e @ x_s + c_b)                        # [2H], W_e = W_attn[:, H:]
    score[s] = w_v . e[s, :] + b_v            (s < L)
    energy   = softmax(score[:L]);  context = energy @ X[:L]

Device strategy: ragged work is packed at 128-position granularity: each
batch's positions form ceil(L/128) sub-chunks; two sub-chunks make one
256-wide device slot.  Same-batch sub-chunks pair first; the per-batch odd
leftovers pair across batches into "mixed" slots, which are placed ONLY at
slot index nchunk-1 — the one slot index the (single, SPMD) program treats
as two independent 128-segments (separate tanh bias / softmax partials /
context rows).  Slots are processed in pairs so the dominant e-matmul runs
as fp8e4 DoubleRow instructions streaming 512 columns (2x PE throughput,
LDWEIGHTS hidden under the stream).  W_e is pre-scaled x512 and X x16 so
fp8 values stay normal; the 1/8192 descale folds into the tanh scale.  The
fp8 quantization error is compensated host-side: a linearized score
correction C[s] = sum_o w_v[o] kappa[o,b] (Delta e)[o,s] (exact residual
algebra against the known quantization errors) is folded into the
per-position mask row.  The c bias columns are computed exactly on host.
Softmax + the context matmul stay fp16/fp32.  Each segment produces
flash-softmax partials (m, Z, ctx) which the host merges exactly.
"""

import numpy as np
import ml_dtypes

import concourse.bass as bass
import concourse.mybir as mybir
import concourse.tile as tile
from concourse import bacc
from concourse.bass_utils import run_bass_kernel_spmd

B, S, H = 16, 2048, 1024
H2 = 2 * H            # 2048 output features / encoder dim
SUB = 128             # packing granularity (one PE partition block)
CHUNK = 256           # sequence positions per device slot
N_CORES = 8
FB2 = H2 // 256       # 8 fp8 DoubleRow f-blocks (256 features each)
OB = H2 // 128        # 16 o-blocks of the output features
NEG = -30000.0        # masked-score offset (exp underflows to exactly 0)
ALPHA = 512.0         # W_e fp8 pre-scale
BETA = 16.0           # X fp8 pre-scale
ISCL = 1.0 / (ALPHA * BETA)

F8 = mybir.dt.float8e4
F16 = mybir.dt.float16
F32 = mybir.dt.float32
NP_F8 = ml_dtypes.float8_e4m3   # TRN fp8_exp4 (max normal +-240)


def build_program(nchunk: int):
    """nchunk must be even; slot nchunk-1 is handled as two 128-segments."""
    nc = bacc.Bacc()

    npairs = nchunk // 2

    xtp_ext = nc.declare_dram_parameter(
        "xtp", [npairs, 128, FB2, 2, 2 * CHUNK], F8, isOutput=False)
    # xc: fp16 X in feature-major layout for the DVE context reduction:
    # xc[slot, p, fb, s] = X[s, fb*128 + p]
    xc_ext = nc.declare_dram_parameter("xc", [nchunk, 128, OB, CHUNK], F16, isOutput=False)
    mask_ext = nc.declare_dram_parameter("mask", [nchunk, CHUNK], F32, isOutput=False)
    # c bias columns, exact from host, per sub-chunk: cb[p, ob, 2*slot+h]
    c_ext = nc.declare_dram_parameter("cb", [128, OB, 2 * nchunk], F32, isOutput=False)
    wet_ext = nc.declare_dram_parameter("wet", [OB, 128, FB2, 2, 128], F8, isOutput=False)
    wv_ext = nc.declare_dram_parameter("wv", [128, OB], F16, isOutput=False)
    # ctxT[oi, p, fb] = context[fb*128 + p] for segment oi
    ctx_out = nc.declare_dram_parameter("out_ctx", [2 * nchunk, 128, OB], F32, isOutput=True)
    mz_out = nc.declare_dram_parameter("out_mz", [2 * nchunk, 2], F32, isOutput=True)

    DR = mybir.MatmulPerfMode.DoubleRow

    def segs_of(i):
        # (col_lo, col_hi, out_idx); slot nchunk-1 is two 128-segments
        if i == nchunk - 1:
            return [(0, 128, 2 * i), (128, 256, 2 * i + 1)]
        return [(0, 256, 2 * i)]

    from contextlib import ExitStack
    with tile.TileContext(nc) as tc, ExitStack() as stk:
        singles = stk.enter_context(tc.tile_pool(name="singles", bufs=1))
        xtp = stk.enter_context(tc.tile_pool(name="xtp", bufs=2))
        xcp = stk.enter_context(tc.tile_pool(name="xcp", bufs=4))
        tp = stk.enter_context(tc.tile_pool(name="tp", bufs=4))
        smalls = stk.enter_context(tc.tile_pool(name="smalls", bufs=3))
        eps = stk.enter_context(tc.tile_pool(name="eps", bufs=3, space="PSUM"))
        sps = stk.enter_context(tc.tile_pool(name="sps", bufs=2, space="PSUM"))

        # resident weights as one tile per o-block; xtp0 lands in fb-quarters
        # so the first e-matmul can start after ~0.5MB of DMA
        wet_sb = []
        c_all_sb = singles.tile([128, OB, 2 * nchunk], F32)
        wv_sb = singles.tile([128, OB], F16)
        mask_sb = singles.tile([1, nchunk, CHUNK], F32)
        xt0_sb = xtp.tile([128, FB2, 2, 2 * CHUNK], F8, tag="xt")
        for ob in range(OB):
            if ob == 0:
                nc.sync.dma_start(out=c_all_sb[:], in_=c_ext[:])
            w1 = singles.tile([128, FB2, 2, 128], F8, tag=f"wet{ob}")
            nc.sync.dma_start(out=w1[:], in_=wet_ext[ob])
            wet_sb.append(w1)
            if ob == 0:
                for q in range(FB2):
                    nc.sync.dma_start(
                        out=xt0_sb[:, q:q + 1, :, :],
                        in_=xtp_ext[0, :, q:q + 1, :, :])
                nc.sync.dma_start(out=wv_sb[:], in_=wv_ext[:])
                nc.sync.dma_start(out=mask_sb[0:1, :, :], in_=mask_ext[:])
        mz_all = singles.tile([1, 2 * nchunk, 2], F32)

        def emit_ctx(i, p_sb, xc_sb):
            # context on GpSimd+DVE: broadcast the energy row, then ONE big
            # multiply (broadcast AP over the 16 f-blocks) + ONE 3D reduce
            # per segment (PE not involved)
            p_bc = smalls.tile([128, CHUNK], F16, tag="pbc", name=f"pbc{i}")
            for (lo, hi, oi) in segs_of(i):
                nc.gpsimd.partition_broadcast(p_bc[:, lo:hi], p_sb[0:1, lo:hi])
                ctxT = smalls.tile([128, OB], F32, tag="ctxT", name=f"ctxT{oi}")
                junk = smalls.tile([128, OB, hi - lo], F16, tag="junk",
                                   name=f"junk{oi}", bufs=2)
                in0, in1 = bass.broadcast_tensor_aps(
                    xc_sb[:, :, lo:hi], p_bc[:, None, lo:hi])
                nc.vector.tensor_tensor(
                    out=junk[:], in0=in0, in1=in1, op=mybir.AluOpType.mult)
                nc.vector.tensor_reduce(
                    out=ctxT[:], in_=junk[:],
                    axis=mybir.AxisListType.X, op=mybir.AluOpType.add,
                )
                nc.sync.dma_start(out=ctx_out[oi], in_=ctxT[:])

        def emit_scores(i, t_sb):
            # scores[s] = sum_o w_v[o] t[o, s], accumulated into ONE PSUM row
            # (each matmul depends only on its ob's tanh)
            s_ps = sps.tile([1, CHUNK], F32, tag="s", bufs=2)
            for ob in range(OB):
                nc.tensor.matmul(
                    s_ps[:],
                    lhsT=wv_sb[:, ob:ob + 1],
                    rhs=t_sb[:, ob, :],
                    start=(ob == 0), stop=(ob == OB - 1),
                )
            return s_ps

        def emit_softmax(i, s_ps):
            # masked softmax partials per segment: fold the 4 partial rows +
            # mask (DVE may read at most one PSUM operand per op)
            p_sb = smalls.tile([1, CHUNK], F16, tag="p")
            for (lo, hi, oi) in segs_of(i):
                w = hi - lo
                sc_sb = smalls.tile([1, w], F32, tag="fold", name=f"fold_{oi}")
                nc.vector.tensor_tensor(
                    out=sc_sb[:], in0=s_ps[0:1, lo:hi], in1=mask_sb[0:1, i, lo:hi],
                    op=mybir.AluOpType.add,
                )
                nc.vector.tensor_reduce(
                    out=mz_all[0:1, oi, 0:1], in_=sc_sb[:],
                    axis=mybir.AxisListType.X, op=mybir.AluOpType.max, negate=True,
                )
                nc.scalar.activation(
                    out=p_sb[0:1, lo:hi], in_=sc_sb[:],
                    func=mybir.ActivationFunctionType.Exp,
                    bias=mz_all[0:1, oi, 0:1], scale=1.0,
                    accum_out=mz_all[0:1, oi, 1:2],
                )
            return p_sb

        for g in range(npairs):
            chunks = [2 * g, 2 * g + 1]
            if g == 0:
                xt_sb = xt0_sb
            else:
                xt_sb = xtp.tile([128, FB2, 2, 2 * CHUNK], F8, tag="xt")
                nc.sync.dma_start(out=xt_sb[:], in_=xtp_ext[g])
            xc_list = []
            for h, i in enumerate(chunks):
                xc_sb = xcp.tile([128, OB, CHUNK], F16, tag="xc", name=f"xc{g}_{h}")
                nc.sync.dma_start(out=xc_sb[:], in_=xc_ext[i])
                xc_list.append(xc_sb)

            t_list = [tp.tile([128, OB, CHUNK], F16, tag="t", name=f"t{g}_{h}")
                      for h in range(2)]
            for ob in range(OB):
                e_ps = eps.tile([128, 2 * CHUNK], F32, tag="e")
                for fb in range(FB2):
                    nc.tensor.matmul(
                        e_ps[:],
                        lhsT=wet_sb[ob][:, fb, :, :],
                        rhs=xt_sb[:, fb, :, :],
                        start=(fb == 0), stop=(fb == FB2 - 1),
                        perf_mode=DR,
                    )
                for h, i in enumerate(chunks):
                    for (lo, hi, oi) in segs_of(i):
                        nc.scalar.activation(
                            out=t_list[h][:, ob, lo:hi],
                            in_=e_ps[:, h * CHUNK + lo:h * CHUNK + hi],
                            func=mybir.ActivationFunctionType.Tanh,
                            bias=c_all_sb[:, ob, oi:oi + 1], scale=ISCL,
                        )

            for h, i in enumerate(chunks):
                s_ps = emit_scores(i, t_list[h])
                p_sb = emit_softmax(i, s_ps)
                emit_ctx(i, p_sb, xc_list[h])

        nc.sync.dma_start(out=mz_out[:], in_=mz_all[0:1, :, :])

    nc.compile()
    return nc


def kernel(encoder_out, hidden, W_attn, b_attn, w_v, b_v, lengths):
    encoder_out = np.asarray(encoder_out)
    hidden = np.asarray(hidden)
    W_attn = np.asarray(W_attn)
    b_attn = np.asarray(b_attn)
    w_v = np.asarray(w_v)
    b_v = np.asarray(b_v)
    lengths = np.asarray(lengths)

    # ---- host-side 128-granular packing from the runtime lengths ----
    # sub-chunk: (batch, s0, v<=128); slot: two sub-chunks
    same_slots = []   # [(sub, sub)] both from one batch
    leftovers = []
    for b in range(B):
        L = int(lengths[b])
        subs = [(b, s0, min(SUB, L - s0)) for s0 in range(0, L, SUB)]
        for j in range(len(subs) // 2):
            same_slots.append((subs[2 * j], subs[2 * j + 1]))
        if len(subs) % 2:
            leftovers.append(subs[-1])
    mixed_slots = []
    while len(leftovers) >= 2 and len(mixed_slots) < N_CORES:
        mixed_slots.append((leftovers.pop(0), leftovers.pop(0)))
    # any remaining leftover becomes a half-empty same-batch slot
    for lf in leftovers:
        same_slots.append((lf, None))

    nslots = len(same_slots) + len(mixed_slots)
    nchunk = max(2, -(-nslots // N_CORES))
    if nchunk % 2:
        nchunk += 1
    # mixed slots only fit at index nchunk-1 (one per core)
    while len(mixed_slots) > N_CORES or \
            len(same_slots) > N_CORES * (nchunk - 1) + (N_CORES - len(mixed_slots)):
        nchunk += 2

    # per-core slot assignment: indices 0..nchunk-2 from same_slots; index
    # nchunk-1 takes mixed first, then remaining same, then padding
    core_slots = [[None] * nchunk for _ in range(N_CORES)]
    it_same = iter(same_slots)
    filled = 0
    for c in range(N_CORES):
        for i in range(nchunk - 1):
            core_slots[c][i] = next(it_same, None)
    for c in range(N_CORES):
        if c < len(mixed_slots):
            core_slots[c][nchunk - 1] = mixed_slots[c]
        else:
            core_slots[c][nchunk - 1] = next(it_same, None)
    assert next(it_same, None) is None

    We = W_attn[:, H:]                          # [2H, 2H]
    Wh = W_attn[:, :H]                          # [2H, H]

    # ---- fp8 weight quantization (+ residual for the score correction) ----
    W8q = (We * ALPHA).astype(NP_F8)            # [o, f] fp8 payload
    W8f = W8q.astype(np.float32)
    dW = We * ALPHA - W8f                       # exact residual (host)

    # wet[ob, p, fb, i, q] = W8[ob*128+q, fb*256 + i*128 + p]
    wet = np.ascontiguousarray(
        W8q.reshape(OB, 128, FB2, 2, 128).transpose(0, 4, 2, 3, 1))
    wv = np.ascontiguousarray(w_v[0].reshape(OB, 128).T).astype(np.float16)

    # ---- linearized fp8 score correction (host, exact residual algebra) ----
    # kappa[o, b] = E_s[1 - tanh^2(e8[o,s])] with e8 ~ N(c[o,b], sigma_o^2)
    c_all = (Wh @ hidden) + b_attn[:, None]     # [2H, B]
    sig = np.linalg.norm(We, axis=1)            # [2H]
    gh_x, gh_w = np.polynomial.hermite_e.hermegauss(8)
    gh_w = (gh_w / gh_w.sum()).astype(np.float64)
    z = c_all[:, None, :] + sig[:, None, None] * gh_x[None, :, None]
    kappa = np.einsum("okb,k->ob", 1.0 / np.cosh(z) ** 2, gh_w,
                      optimize=True).astype(np.float32)   # [2H, B]
    wk = w_v[0][:, None] * kappa                # [2H_o, B]
    g_all = dW.T @ wk                           # [2H_f, B]
    h8_all = W8f.T @ wk                         # [2H_f, B]

    x16 = encoder_out.astype(np.float16)
    x8 = np.empty((B, S, H2), NP_F8)
    corr = np.empty((B, S), np.float32)
    for b in range(B):
        x8[b] = (encoder_out[b] * BETA).astype(NP_F8)
        x8f = x8[b].astype(np.float32)          # [S, 2H]
        dX = encoder_out[b] * BETA - x8f
        corr[b] = (BETA * (encoder_out[b] @ g_all[:, b])
                   + dX @ h8_all[:, b]) * ISCL

    nc = build_program(nchunk)

    npairs = nchunk // 2
    c_dev = c_all.reshape(OB, 128, B)           # [ob, p, B]

    # segment records for the host flash merge: (batch, core, out_idx)
    seg_recs = []
    in_maps = []
    for c in range(N_CORES):
        xtp_a = np.zeros((npairs, 128, FB2, 2, 2 * CHUNK), NP_F8)
        xc = np.zeros((nchunk, 128, OB, CHUNK), np.float16)
        mask = np.full((nchunk, CHUNK), NEG + float(b_v[0]), np.float32)
        cb = np.zeros((128, OB, 2 * nchunk), np.float32)
        for i in range(nchunk):
            slot = core_slots[c][i]
            if slot is None:
                continue
            two_seg = (i == nchunk - 1)
            for h, sub in enumerate(slot):
                if sub is None:
                    continue
                b, s0, v = sub
                chunk8 = x8[b, s0:s0 + v, :]                 # [v, 2048] fp8
                xt_block = np.zeros((128, FB2, 2, SUB), NP_F8)
                xt_block[:, :, :, :v] = chunk8.reshape(v, FB2, 2, 128).transpose(3, 1, 2, 0)
                xtp_a[i // 2, :, :, :, (i % 2) * CHUNK + h * SUB:
                      (i % 2) * CHUNK + (h + 1) * SUB] = xt_block
                full = np.zeros((SUB, H2), np.float16)
                full[:v] = x16[b, s0:s0 + v, :]
                # xc[p, fb, s] = X[s, fb*128 + p]
                xc[i, :, :, h * SUB:h * SUB + SUB] = \
                    full.reshape(SUB, OB, 128).transpose(2, 1, 0)
                mask[i, h * SUB:h * SUB + v] = float(b_v[0]) + corr[b, s0:s0 + v]
                cb[:, :, 2 * i + h] = c_dev[:, :, b].T
            if two_seg:
                for h, sub in enumerate(slot):
                    if sub is not None:
                        seg_recs.append((sub[0], c, 2 * i + h))
            else:
                b0 = slot[0][0]
                cb[:, :, 2 * i + 1] = c_dev[:, :, b0].T  # unused but sane
                seg_recs.append((b0, c, 2 * i))
        in_maps.append(dict(xtp=xtp_a, xc=xc, mask=mask, cb=cb, wet=wet, wv=wv))

    def run_once():
        res = run_bass_kernel_spmd(nc, in_maps, core_ids=list(range(N_CORES)))
        negm = np.stack([res.results[c]["out_mz"][:, 0] for c in range(N_CORES)])
        zz = np.stack([res.results[c]["out_mz"][:, 1] for c in range(N_CORES)])
        ctx = np.stack([res.results[c]["out_ctx"] for c in range(N_CORES)])
        return negm, zz, ctx

    def merge(parts):
        negm, zz, ctx = parts
        # ---- exact flash-softmax merge on host ----
        out = np.zeros((B, H2), np.float32)
        ok = True
        for b in range(B):
            idxs = [(c, oi) for (ub, c, oi) in seg_recs if ub == b]
            ms = np.array([-float(negm[c, oi]) for c, oi in idxs])
            if not np.isfinite(ms).all():
                ok = False
                ms = np.nan_to_num(ms, nan=-np.inf)
            m = ms.max()
            w = np.exp(ms - m)
            Z = float(sum(wi * float(zz[c, oi]) for wi, (c, oi) in zip(w, idxs)))
            if not (Z > 0 and np.isfinite(Z)):
                ok = False
                Z = 1.0
            acc = np.zeros(H2, np.float64)
            for wi, (c, oi) in zip(w, idxs):
                # ctx layout [128 p, OB fb]: feature f = fb*128 + p
                acc += wi * ctx[c, oi].T.reshape(H2).astype(np.float64)
            out[b] = (acc / Z).astype(np.float32)
        # context rows are convex combinations of encoder_out rows
        ok = ok and np.isfinite(out).all() and np.abs(out).max() < 50.0
        return out, ok

    out, ok = merge(run_once())
    if not ok:  # one retry on gross corruption
        out, ok = merge(run_once())
    return out
